# revision 25
# baseline (speedup 1.0000x reference)
"""Trainium2 Bass kernel for the MoE block (nn_MoEBlock_5592047420171).

Strategy: data-parallel over tokens across 8 NeuronCores (1024 tokens/core,
all weights replicated; no collectives).  Layout: d_ff on partitions, tokens
on the free dim.

  out[t,:] = v_t * (relu(base_t + bi + delta_{e1,t}) + relu(base_t + bi +
             delta_{e2,t})) @ wo^T + 2 v_t * bo

Design (v3):
  * router logits in bf16 (f32 PSUM accum); exp with gate_b as ACT bias in
    [8,T] layout; top-2 / one-hots via gpsimd partition_all_reduce(max) and
    DVE is_equal -- no PE transposes.  Processed in two token halves so the
    whole chain pipelines across engines; masks expanded to the concat-lora
    layout by a DRAM round-trip gather (baseline idiom).
  * lora path bf16: tA = hs@lA_cat (one [128,T] concat matmul), masked per
    top-2 choice by a DVE multiply straight out of PSUM; delta = lB_cat@tA_k
    is one K=128 matmul per choice per tile.
  * base = hs@wi^T in bf16, drained to bf16 by ACT copy (no bias); relu with
    bi as per-partition ACT bias; H = r1+r2 (DVE); v-scaling moved to the
    output drain: out = (wo_psum + 2*bo) * V (one DVE op per out tile).
  * main loop software-pipelined: deltas+combine lag LAG tiles behind base
    matmuls so the router/mask chain hides under base compute and the PE
    never idles (HAM throttling avoided).  wo tiles prefetched during the
    main-loop tail, streamed per-fc (re-streamed per token-half).
"""

import numpy as np
from contextlib import ExitStack

import concourse.bass as bass
import concourse.tile as tile
from concourse import bacc, mybir, bass_isa
from concourse.bass_utils import run_bass_kernel_spmd

B, S, DM, FF, E, RK = 4, 2048, 1024, 4096, 8, 16
NCORES = 8
TOK = B * S            # 8192 tokens
T = TOK // NCORES      # 1024 tokens per core
TT = 512               # token tile width (free dim of big matmuls)
NTT = T // TT          # 2 token tiles
DCH = 8                # d_model chunks of 128
FCH = FF // 128        # 32 d_ff chunks of 128
LAG = 8                # pipeline lag (tiles) between base and delta stages
WO_PRE = 8             # wo tiles prefetched during the main loop

F32 = mybir.dt.float32
BF16 = mybir.dt.bfloat16
ALU = mybir.AluOpType
AF = mybir.ActivationFunctionType
ROP = bass_isa.ReduceOp


def build_bass():
    nc = bacc.Bacc("TRN2", target_bir_lowering=False)

    hsB = nc.declare_dram_parameter("hsB", [128, DCH, T], BF16, isOutput=False)
    gwB = nc.declare_dram_parameter("gwB", [128, DCH, 8], BF16, isOutput=False)
    gbC = nc.declare_dram_parameter("gbC", [8, 1], F32, isOutput=False)
    wiB = nc.declare_dram_parameter("wiB", [FCH, 128, DCH, 128], BF16, isOutput=False)
    biC = nc.declare_dram_parameter("biC", [128, FCH], F32, isOutput=False)
    lAc = nc.declare_dram_parameter("lAc", [128, DCH, 128], BF16, isOutput=False)
    lBc = nc.declare_dram_parameter("lBc", [FCH, 128, 128], BF16, isOutput=False)
    woB = nc.declare_dram_parameter("woB", [FCH, 128, DM], BF16, isOutput=False)
    bo2 = nc.declare_dram_parameter("bo2", [128, 8], F32, isOutput=False)
    gG = nc.declare_dram_parameter("gG", [8, 128], BF16, isOutput=False)
    outT = nc.declare_dram_parameter("outT", [DM, T], BF16, isOutput=True)

    hsB, gwB, gbC, wiB, biC, lAc, lBc, woB, bo2, gG, outT = (
        h.ap() for h in (hsB, gwB, gbC, wiB, biC, lAc, lBc, woB, bo2, gG, outT))

    with tile.TileContext(nc) as tc, ExitStack() as ctx:
        persist = ctx.enter_context(tc.tile_pool(name="persist", bufs=1))

        # ---- resident tensors (DMA order = priority order) ----
        gw_sb = persist.tile([128, DCH, 8], BF16, tag="gw")
        nc.sync.dma_start(out=gw_sb, in_=gwB)
        gb_sb = persist.tile([8, 1], F32, tag="gb")
        nc.sync.dma_start(out=gb_sb, in_=gbC)
        hsB_sb = persist.tile([128, DCH, T], BF16, tag="hsB")
        for h in range(NTT):
            hsl = slice(h * TT, (h + 1) * TT)
            nc.sync.dma_start(out=hsB_sb[:, :, hsl], in_=hsB[:, :, hsl])
        lA_sb = persist.tile([128, DCH, 128], BF16, tag="lA")
        nc.sync.dma_start(out=lA_sb, in_=lAc)
        bi_sb = persist.tile([128, FCH], F32, tag="bi")
        nc.sync.dma_start(out=bi_sb, in_=biC)
        bo2_sb = persist.tile([128, 8], F32, tag="bo2")
        nc.sync.dma_start(out=bo2_sb, in_=bo2)
        gG_sb = persist.tile([8, 128], BF16, tag="gG")
        nc.sync.dma_start(out=gG_sb, in_=gG)

        # ---- router state (SBUF) ----
        P_sb = persist.tile([8, T], F32, tag="P")
        m1_sb = persist.tile([8, T], F32, tag="m1")
        q_sb = persist.tile([8, T], F32, tag="q")
        m2_sb = persist.tile([8, T], F32, tag="m2")
        S_sb = persist.tile([8, T], F32, tag="S")
        oh1_sb = persist.tile([8, T], BF16, tag="oh1")
        oh2_sb = persist.tile([8, T], BF16, tag="oh2")
        rv_sb = persist.tile([1, T], F32, tag="rv")
        vn_sb = persist.tile([1, T], F32, tag="vn")
        v_sb = persist.tile([1, T], F32, tag="v")
        Vb_sb = persist.tile([128, T], F32, tag="Vb")
        M1_sb = persist.tile([128, T], BF16, tag="M1")
        M2_sb = persist.tile([128, T], BF16, tag="M2")
        tA1_sb = persist.tile([128, T], BF16, tag="tA1")
        tA2_sb = persist.tile([128, T], BF16, tag="tA2")
        H_sb = [persist.tile([128, T], BF16, tag=f"H{fc}", name=f"H{fc}")
                for fc in range(FCH)]

        wo_pool = ctx.enter_context(tc.tile_pool(name="wo_sb", bufs=WO_PRE + 2))
        wo_tiles = {}

        def wo_fetch(tt, fc):
            wo_t = wo_pool.tile([128, DM], BF16, tag="wo", name=f"wo{tt}_{fc}")
            nc.sync.dma_start(out=wo_t, in_=woB[fc])
            wo_tiles[(tt, fc)] = wo_t

        with tc.tile_pool(name="base_ps", bufs=2, space="PSUM") as base_ps:
            # ---- phase 1: router + lora-A, pipelined per token half ----
            with (
                tc.tile_pool(name="rt_ps", bufs=1, space="PSUM") as rt_ps,
                tc.tile_pool(name="la_ps", bufs=2, space="PSUM") as la_ps,
                tc.tile_pool(name="msk_ps", bufs=2, space="PSUM") as msk_ps,
            ):
                for h in range(NTT):
                    hsl = slice(h * TT, (h + 1) * TT)
                    lg = rt_ps.tile([8, TT], F32, tag="lg")
                    for ci in range(DCH):
                        nc.tensor.matmul(
                            lg, lhsT=gw_sb[:, ci, :], rhs=hsB_sb[:, ci, hsl],
                            start=(ci == 0), stop=(ci == DCH - 1))
                    nc.scalar.activation(P_sb[:, hsl], lg, AF.Exp, bias=gb_sb)
                    # top-1/top-2 via partition all-reduce max + is_equal
                    nc.gpsimd.partition_all_reduce(
                        m1_sb[:, hsl], P_sb[:, hsl], channels=8,
                        reduce_op=ROP.max)
                    nc.vector.tensor_tensor(
                        out=oh1_sb[:, hsl], in0=P_sb[:, hsl],
                        in1=m1_sb[:, hsl], op=ALU.is_equal)
                    nc.vector.scalar_tensor_tensor(
                        out=q_sb[:, hsl], in0=oh1_sb[:, hsl], scalar=-1e30,
                        in1=P_sb[:, hsl], op0=ALU.mult, op1=ALU.add)
                    nc.gpsimd.partition_all_reduce(
                        m2_sb[:, hsl], q_sb[:, hsl], channels=8,
                        reduce_op=ROP.max)
                    nc.vector.tensor_tensor(
                        out=oh2_sb[:, hsl], in0=q_sb[:, hsl],
                        in1=m2_sb[:, hsl], op=ALU.is_equal)
                    # expand one-hots to concat-row masks on the PE:
                    # M = G @ oh where G replicates expert row e to rows
                    # 16e..16e+15 (exact for 0/1 values)
                    M1p = msk_ps.tile([128, TT], F32, tag="Mp", name=f"M1{h}")
                    nc.tensor.matmul(M1p, lhsT=gG_sb, rhs=oh1_sb[:, hsl],
                                     start=True, stop=True)
                    nc.scalar.copy(out=M1_sb[:, hsl], in_=M1p)
                    M2p = msk_ps.tile([128, TT], F32, tag="Mp", name=f"M2{h}")
                    nc.tensor.matmul(M2p, lhsT=gG_sb, rhs=oh2_sb[:, hsl],
                                     start=True, stop=True)
                    nc.scalar.copy(out=M2_sb[:, hsl], in_=M2p)
                    # lora-A for this half + per-choice masking
                    pta = la_ps.tile([128, TT], F32, tag="pta")
                    for ci in range(DCH):
                        nc.tensor.matmul(
                            pta, lhsT=lA_sb[:, ci, :], rhs=hsB_sb[:, ci, hsl],
                            start=(ci == 0), stop=(ci == DCH - 1))
                    nc.vector.tensor_tensor(
                        out=tA1_sb[:, hsl], in0=pta, in1=M1_sb[:, hsl],
                        op=ALU.mult)
                    nc.vector.tensor_tensor(
                        out=tA2_sb[:, hsl], in0=pta, in1=M2_sb[:, hsl],
                        op=ALU.mult)

                # val_sum v = (m1 + m2) / sum(P): only needed by the phase-3
                # drains -- hint the scheduler to keep it off the router
                # critical path (the [1,T] reciprocal alone is ~6.5us on DVE)
                with tc.tile_wait_until(0.08):
                    nc.gpsimd.partition_all_reduce(S_sb, P_sb, channels=8,
                                                   reduce_op=ROP.add)
                    nc.vector.reciprocal_approx_fast(rv_sb, S_sb[0:1, :])
                    nc.vector.tensor_tensor(out=vn_sb, in0=m1_sb[0:1, :],
                                            in1=m2_sb[0:1, :], op=ALU.add)
                    nc.vector.tensor_tensor(out=v_sb, in0=vn_sb,
                                            in1=rv_sb, op=ALU.mult)
                    nc.gpsimd.partition_broadcast(Vb_sb, v_sb, channels=128)

            # ---- phase 2: main loop (base + delta + combine), pipelined ----
            with (
                tc.tile_pool(name="d_ps", bufs=4, space="PSUM") as d_ps,
                tc.tile_pool(name="wi_sb", bufs=6) as wi_pool,
                tc.tile_pool(name="lb_sb", bufs=8) as lb_pool,
                tc.tile_pool(name="bs_sb", bufs=LAG + 3) as bs_pool,
                tc.tile_pool(name="s_sb", bufs=3) as s_pool,
                tc.tile_pool(name="r_sb", bufs=3) as r_pool,
            ):
                tiles = [(fc, tt) for fc in range(FCH) for tt in range(NTT)]
                wi_cur = None
                lb_tiles = {}
                pend = {}
                for step in range(len(tiles) + LAG):
                    if step < len(tiles):
                        fc, tt = tiles[step]
                        tsl = slice(tt * TT, (tt + 1) * TT)
                        if tt == 0:
                            wi_cur = wi_pool.tile([128, DCH, 128], BF16,
                                                  tag="wi", name=f"wi{fc}")
                            nc.sync.dma_start(out=wi_cur, in_=wiB[fc])
                            lb_cur = lb_pool.tile([128, 128], BF16,
                                                  tag="lb", name=f"lb{fc}")
                            nc.sync.dma_start(out=lb_cur, in_=lBc[fc])
                            lb_tiles[fc] = lb_cur
                        bps = base_ps.tile([128, TT], F32, tag="bps")
                        for ci in range(DCH):
                            nc.tensor.matmul(
                                bps, lhsT=wi_cur[:, ci, :],
                                rhs=hsB_sb[:, ci, tsl],
                                start=(ci == 0), stop=(ci == DCH - 1))
                        bs = bs_pool.tile([128, TT], BF16, tag="bs")
                        nc.scalar.copy(out=bs, in_=bps)
                        pend[step] = (fc, tt, tsl, bs)
                        if step == 44:
                            for fcp in range(WO_PRE):
                                wo_fetch(0, fcp)
                    j = step - LAG
                    if j in pend:
                        fcj, ttj, tslj, bsj = pend.pop(j)
                        bic = bi_sb[:, fcj:fcj + 1]
                        d1 = d_ps.tile([128, TT], F32, tag="d")
                        nc.tensor.matmul(d1, lhsT=lb_tiles[fcj],
                                         rhs=tA1_sb[:, tslj],
                                         start=True, stop=True)
                        d2 = d_ps.tile([128, TT], F32, tag="d")
                        nc.tensor.matmul(d2, lhsT=lb_tiles[fcj],
                                         rhs=tA2_sb[:, tslj],
                                         start=True, stop=True)
                        s1 = s_pool.tile([128, TT], BF16, tag="s")
                        nc.vector.tensor_tensor(out=s1, in0=d1, in1=bsj,
                                                op=ALU.add)
                        r1 = r_pool.tile([128, TT], BF16, tag="r")
                        nc.scalar.activation(r1, s1, AF.Relu, bias=bic)
                        s2 = s_pool.tile([128, TT], BF16, tag="s")
                        nc.vector.tensor_tensor(out=s2, in0=d2, in1=bsj,
                                                op=ALU.add)
                        r2 = r_pool.tile([128, TT], BF16, tag="r")
                        nc.scalar.activation(r2, s2, AF.Relu, bias=bic)
                        nc.vector.tensor_tensor(
                            out=H_sb[fcj][:, tslj], in0=r1, in1=r2, op=ALU.add)

        # ---- phase 3: wo matmul, out = (psum + 2*bo) * v ----
        with (
            tc.tile_pool(name="wo_ps", bufs=8, space="PSUM") as wo_ps,
            tc.tile_pool(name="o_sb", bufs=3) as o_pool,
        ):
            for tt in range(NTT):
                tsl = slice(tt * TT, (tt + 1) * TT)
                ops = [wo_ps.tile([128, TT], F32, tag="wops",
                                  name=f"o{tt}_{dc}") for dc in range(DCH)]
                for fc in range(FCH):
                    if (tt, fc) not in wo_tiles:
                        wo_fetch(tt, fc)
                    wo_t = wo_tiles.pop((tt, fc))
                    if tt == 0 and fc == 28:
                        for fcp in range(4):
                            wo_fetch(1, fcp)
                    for dc in range(DCH):
                        nc.tensor.matmul(
                            ops[dc], lhsT=wo_t[:, dc * 128:(dc + 1) * 128],
                            rhs=H_sb[fc][:, tsl],
                            start=(fc == 0), stop=(fc == FCH - 1))
                for dc in range(DCH):
                    # split drain: ACT does psum+2bo (per-partition bias) to
                    # bf16, DVE does the per-token *v -- halves the end tail
                    o_a = o_pool.tile([128, TT], BF16, tag="oa")
                    nc.scalar.activation(o_a, ops[dc], AF.Identity,
                                         bias=bo2_sb[:, dc:dc + 1])
                    o_t = o_pool.tile([128, TT], BF16, tag="ot")
                    nc.vector.tensor_tensor(out=o_t, in0=o_a,
                                            in1=Vb_sb[:, tsl], op=ALU.mult)
                    nc.sync.dma_start(
                        out=outT[dc * 128:(dc + 1) * 128, tsl], in_=o_t)

    nc.compile()
    return nc


def prep_inputs(hidden_states, wi, bi, wo, bo, lora_A, lora_B, gate_w, gate_b):
    """Host-side layout prep; returns per-core input maps."""
    import ml_dtypes
    bf = ml_dtypes.bfloat16
    f32 = np.float32
    hs = np.asarray(hidden_states, f32).reshape(TOK, DM)
    wi = np.asarray(wi, f32); bi = np.asarray(bi, f32)
    wo = np.asarray(wo, f32); bo = np.asarray(bo, f32)
    lora_A = np.asarray(lora_A, f32); lora_B = np.asarray(lora_B, f32)
    gate_w = np.asarray(gate_w, f32); gate_b = np.asarray(gate_b, f32)

    # wi^T in (fc, k, ci, m) bf16 chunks
    wiB = np.ascontiguousarray(
        wi.T.reshape(DCH, 128, FCH, 128).transpose(2, 1, 0, 3)).astype(bf)
    biC = np.ascontiguousarray(bi.reshape(FCH, 128).T)

    # lora-A concatenated: columns 16e+r = lora_A[e,r,:]
    lA_cat = np.concatenate([lora_A[e].T for e in range(E)], axis=1)  # [DM,128]
    lAc = np.ascontiguousarray(
        lA_cat.reshape(DCH, 128, 128).transpose(1, 0, 2)).astype(bf)
    # lora-B concatenated: rows 16e+r = lora_B[e,:,r]
    lB_cat = np.concatenate([lora_B[e].T for e in range(E)], axis=0)  # [128,FF]
    lBc = np.ascontiguousarray(
        lB_cat.reshape(128, FCH, 128).transpose(1, 0, 2)).astype(bf)

    gwB = np.ascontiguousarray(
        gate_w.T.reshape(DCH, 128, E).transpose(1, 0, 2)).astype(bf)
    gbC = np.ascontiguousarray(gate_b.reshape(E, 1))

    woB = np.ascontiguousarray(wo.T.reshape(FCH, 128, DM)).astype(bf)
    bo2 = np.ascontiguousarray((2.0 * bo).reshape(DCH, 128).T)       # [128,8]
    gG = np.kron(np.eye(E, dtype=f32), np.ones((1, 16), f32)).astype(bf)

    shared = dict(wiB=wiB, biC=biC, lAc=lAc, lBc=lBc, gwB=gwB, gbC=gbC,
                  woB=woB, bo2=bo2, gG=gG)
    in_maps = []
    for c in range(NCORES):
        hsc = hs[c * T:(c + 1) * T]                                  # [T, DM]
        hsB = np.ascontiguousarray(
            hsc.T.reshape(DCH, 128, T).transpose(1, 0, 2)).astype(bf)
        in_maps.append(dict(hsB=hsB, **shared))
    return in_maps


def run(in_maps, **kwargs):
    nc = build_bass()
    return nc, run_bass_kernel_spmd(nc, in_maps, list(range(NCORES)), **kwargs)


def kernel(hidden_states, wi, bi, wo, bo, lora_A, lora_B, gate_w, gate_b):
    in_maps = prep_inputs(hidden_states, wi, bi, wo, bo, lora_A, lora_B,
                          gate_w, gate_b)
    _, res = run(in_maps)
    out = np.stack([res.results[c]["outT"].T for c in range(NCORES)])
    return out.reshape(B, S, DM).astype(np.float32)


# revision 26
# speedup vs baseline: 1.0279x; 1.0279x over previous
"""Trainium2 Bass kernel for the MoE block (nn_MoEBlock_5592047420171).

Strategy: data-parallel over tokens across 8 NeuronCores (1024 tokens/core,
all weights replicated; no collectives).  Layout: d_ff on partitions, tokens
on the free dim.

  out[t,:] = v_t * (relu(base_t + bi + delta_{e1,t}) + relu(base_t + bi +
             delta_{e2,t})) @ wo^T + 2 v_t * bo

Design (v3):
  * router logits in bf16 (f32 PSUM accum); exp with gate_b as ACT bias in
    [8,T] layout; top-2 / one-hots via gpsimd partition_all_reduce(max) and
    DVE is_equal -- no PE transposes.  Processed in two token halves so the
    whole chain pipelines across engines; masks expanded to the concat-lora
    layout by a DRAM round-trip gather (baseline idiom).
  * lora path bf16: tA = hs@lA_cat (one [128,T] concat matmul), masked per
    top-2 choice by a DVE multiply straight out of PSUM; delta = lB_cat@tA_k
    is one K=128 matmul per choice per tile.
  * base = hs@wi^T in bf16, drained to bf16 by ACT copy (no bias); relu with
    bi as per-partition ACT bias; H = r1+r2 (DVE); v-scaling moved to the
    output drain: out = (wo_psum + 2*bo) * V (one DVE op per out tile).
  * main loop software-pipelined: deltas+combine lag LAG tiles behind base
    matmuls so the router/mask chain hides under base compute and the PE
    never idles (HAM throttling avoided).  wo tiles prefetched during the
    main-loop tail, streamed per-fc (re-streamed per token-half).
"""

import numpy as np
from contextlib import ExitStack

import concourse.bass as bass
import concourse.tile as tile
from concourse import bacc, mybir, bass_isa
from concourse.bass_utils import run_bass_kernel_spmd

B, S, DM, FF, E, RK = 4, 2048, 1024, 4096, 8, 16
NCORES = 8
TOK = B * S            # 8192 tokens
T = TOK // NCORES      # 1024 tokens per core
TT = 512               # token tile width (free dim of big matmuls)
NTT = T // TT          # 2 token tiles
DCH = 8                # d_model chunks of 128
FCH = FF // 128        # 32 d_ff chunks of 128
LAG = 8                # pipeline lag (tiles) between base and delta stages
WO_PRE = 8             # wo tiles prefetched during the main loop

F32 = mybir.dt.float32
BF16 = mybir.dt.bfloat16
ALU = mybir.AluOpType
AF = mybir.ActivationFunctionType
ROP = bass_isa.ReduceOp


def build_bass():
    nc = bacc.Bacc("TRN2", target_bir_lowering=False)

    hsB = nc.declare_dram_parameter("hsB", [128, DCH, T], BF16, isOutput=False)
    gwB = nc.declare_dram_parameter("gwB", [128, DCH, 8], BF16, isOutput=False)
    gbC = nc.declare_dram_parameter("gbC", [8, 1], F32, isOutput=False)
    wiB = nc.declare_dram_parameter("wiB", [FCH, 128, DCH, 128], BF16, isOutput=False)
    biC = nc.declare_dram_parameter("biC", [128, FCH], F32, isOutput=False)
    lAc = nc.declare_dram_parameter("lAc", [128, DCH, 128], BF16, isOutput=False)
    lBc = nc.declare_dram_parameter("lBc", [FCH, 128, 128], BF16, isOutput=False)
    woB = nc.declare_dram_parameter("woB", [FCH, 128, DM], BF16, isOutput=False)
    bo2 = nc.declare_dram_parameter("bo2", [128, 8], F32, isOutput=False)
    gG = nc.declare_dram_parameter("gG", [8, 128], BF16, isOutput=False)
    outT = nc.declare_dram_parameter("outT", [DM, T], BF16, isOutput=True)

    hsB, gwB, gbC, wiB, biC, lAc, lBc, woB, bo2, gG, outT = (
        h.ap() for h in (hsB, gwB, gbC, wiB, biC, lAc, lBc, woB, bo2, gG, outT))

    with tile.TileContext(nc) as tc, ExitStack() as ctx:
        persist = ctx.enter_context(tc.tile_pool(name="persist", bufs=1))

        # ---- resident tensors (DMA order = priority order) ----
        gw_sb = persist.tile([128, DCH, 8], BF16, tag="gw")
        nc.sync.dma_start(out=gw_sb, in_=gwB)
        gb_sb = persist.tile([8, 1], F32, tag="gb")
        nc.sync.dma_start(out=gb_sb, in_=gbC)
        hsB_sb = persist.tile([128, DCH, T], BF16, tag="hsB")
        for h in range(NTT):
            hsl = slice(h * TT, (h + 1) * TT)
            nc.sync.dma_start(out=hsB_sb[:, :, hsl], in_=hsB[:, :, hsl])
        lA_sb = persist.tile([128, DCH, 128], BF16, tag="lA")
        nc.sync.dma_start(out=lA_sb, in_=lAc)
        bi_sb = persist.tile([128, FCH], F32, tag="bi")
        nc.sync.dma_start(out=bi_sb, in_=biC)
        bo2_sb = persist.tile([128, 8], F32, tag="bo2")
        nc.sync.dma_start(out=bo2_sb, in_=bo2)
        gG_sb = persist.tile([8, 128], BF16, tag="gG")
        nc.sync.dma_start(out=gG_sb, in_=gG)

        # ---- router state (SBUF) ----
        P_sb = persist.tile([8, T], F32, tag="P")
        m1_sb = persist.tile([8, T], F32, tag="m1")
        q_sb = persist.tile([8, T], F32, tag="q")
        m2_sb = persist.tile([8, T], F32, tag="m2")
        S_sb = persist.tile([8, T], F32, tag="S")
        oh1_sb = persist.tile([8, T], BF16, tag="oh1")
        oh2_sb = persist.tile([8, T], BF16, tag="oh2")
        rv_sb = persist.tile([1, T], F32, tag="rv")
        vn_sb = persist.tile([1, T], F32, tag="vn")
        v_sb = persist.tile([1, T], F32, tag="v")
        Vb_sb = persist.tile([128, T], F32, tag="Vb")
        M1_sb = persist.tile([128, T], BF16, tag="M1")
        M2_sb = persist.tile([128, T], BF16, tag="M2")
        tA1_sb = persist.tile([128, T], BF16, tag="tA1")
        tA2_sb = persist.tile([128, T], BF16, tag="tA2")
        H_sb = [persist.tile([128, T], BF16, tag=f"H{fc}", name=f"H{fc}")
                for fc in range(FCH)]

        wo_pool = ctx.enter_context(tc.tile_pool(name="wo_sb", bufs=WO_PRE + 2))
        wo_tiles = {}

        def wo_fetch(tt, fc):
            wo_t = wo_pool.tile([128, DM], BF16, tag="wo", name=f"wo{tt}_{fc}")
            nc.sync.dma_start(out=wo_t, in_=woB[fc])
            wo_tiles[(tt, fc)] = wo_t

        with tc.tile_pool(name="base_ps", bufs=2, space="PSUM") as base_ps:
            # ---- phase 1: router + lora-A, pipelined per token half ----
            with (
                tc.tile_pool(name="rt_ps", bufs=1, space="PSUM") as rt_ps,
                tc.tile_pool(name="la_ps", bufs=2, space="PSUM") as la_ps,
                tc.tile_pool(name="msk_ps", bufs=2, space="PSUM") as msk_ps,
            ):
                for h in range(NTT):
                    hsl = slice(h * TT, (h + 1) * TT)
                    lg = rt_ps.tile([8, TT], F32, tag="lg")
                    for ci in range(DCH):
                        nc.tensor.matmul(
                            lg, lhsT=gw_sb[:, ci, :], rhs=hsB_sb[:, ci, hsl],
                            start=(ci == 0), stop=(ci == DCH - 1))
                    nc.scalar.activation(P_sb[:, hsl], lg, AF.Exp, bias=gb_sb)
                    # top-1/top-2 via partition all-reduce max + is_equal
                    nc.gpsimd.partition_all_reduce(
                        m1_sb[:, hsl], P_sb[:, hsl], channels=8,
                        reduce_op=ROP.max)
                    nc.vector.tensor_tensor(
                        out=oh1_sb[:, hsl], in0=P_sb[:, hsl],
                        in1=m1_sb[:, hsl], op=ALU.is_equal)
                    nc.vector.scalar_tensor_tensor(
                        out=q_sb[:, hsl], in0=oh1_sb[:, hsl], scalar=-1e30,
                        in1=P_sb[:, hsl], op0=ALU.mult, op1=ALU.add)
                    nc.gpsimd.partition_all_reduce(
                        m2_sb[:, hsl], q_sb[:, hsl], channels=8,
                        reduce_op=ROP.max)
                    nc.vector.tensor_tensor(
                        out=oh2_sb[:, hsl], in0=q_sb[:, hsl],
                        in1=m2_sb[:, hsl], op=ALU.is_equal)
                    # expand one-hots to concat-row masks on the PE:
                    # M = G @ oh where G replicates expert row e to rows
                    # 16e..16e+15 (exact for 0/1 values)
                    # wait-hint: keep these tiny PE matmuls from being
                    # scheduled before the one-hot chain has resolved (they
                    # would stall the in-order PE stream)
                    with tc.tile_wait_until(0.028 + 0.004 * h):
                        M1p = msk_ps.tile([128, TT], F32, tag="Mp",
                                          name=f"M1{h}")
                        nc.tensor.matmul(M1p, lhsT=gG_sb, rhs=oh1_sb[:, hsl],
                                         start=True, stop=True)
                        nc.scalar.copy(out=M1_sb[:, hsl], in_=M1p)
                        M2p = msk_ps.tile([128, TT], F32, tag="Mp",
                                          name=f"M2{h}")
                        nc.tensor.matmul(M2p, lhsT=gG_sb, rhs=oh2_sb[:, hsl],
                                         start=True, stop=True)
                        nc.scalar.copy(out=M2_sb[:, hsl], in_=M2p)
                    # lora-A for this half + per-choice masking
                    pta = la_ps.tile([128, TT], F32, tag="pta")
                    for ci in range(DCH):
                        nc.tensor.matmul(
                            pta, lhsT=lA_sb[:, ci, :], rhs=hsB_sb[:, ci, hsl],
                            start=(ci == 0), stop=(ci == DCH - 1))
                    nc.vector.tensor_tensor(
                        out=tA1_sb[:, hsl], in0=pta, in1=M1_sb[:, hsl],
                        op=ALU.mult)
                    nc.vector.tensor_tensor(
                        out=tA2_sb[:, hsl], in0=pta, in1=M2_sb[:, hsl],
                        op=ALU.mult)

                # val_sum v = (m1 + m2) / sum(P): only needed by the phase-3
                # drains -- hint the scheduler to keep it off the router
                # critical path (the [1,T] reciprocal alone is ~6.5us on DVE)
                with tc.tile_wait_until(0.08):
                    nc.gpsimd.partition_all_reduce(S_sb, P_sb, channels=8,
                                                   reduce_op=ROP.add)
                    nc.vector.reciprocal_approx_fast(rv_sb, S_sb[0:1, :])
                    nc.vector.tensor_tensor(out=vn_sb, in0=m1_sb[0:1, :],
                                            in1=m2_sb[0:1, :], op=ALU.add)
                    nc.vector.tensor_tensor(out=v_sb, in0=vn_sb,
                                            in1=rv_sb, op=ALU.mult)
                    nc.gpsimd.partition_broadcast(Vb_sb, v_sb, channels=128)

            # ---- phase 2: main loop (base + delta + combine), pipelined ----
            with (
                tc.tile_pool(name="d_ps", bufs=4, space="PSUM") as d_ps,
                tc.tile_pool(name="wi_sb", bufs=6) as wi_pool,
                tc.tile_pool(name="lb_sb", bufs=8) as lb_pool,
                tc.tile_pool(name="bs_sb", bufs=LAG + 3) as bs_pool,
                tc.tile_pool(name="s_sb", bufs=3) as s_pool,
                tc.tile_pool(name="r_sb", bufs=3) as r_pool,
            ):
                tiles = [(fc, tt) for fc in range(FCH) for tt in range(NTT)]
                wi_cur = None
                lb_tiles = {}
                pend = {}
                for step in range(len(tiles) + LAG):
                    if step < len(tiles):
                        fc, tt = tiles[step]
                        tsl = slice(tt * TT, (tt + 1) * TT)
                        if tt == 0:
                            wi_cur = wi_pool.tile([128, DCH, 128], BF16,
                                                  tag="wi", name=f"wi{fc}")
                            nc.sync.dma_start(out=wi_cur, in_=wiB[fc])
                            lb_cur = lb_pool.tile([128, 128], BF16,
                                                  tag="lb", name=f"lb{fc}")
                            nc.sync.dma_start(out=lb_cur, in_=lBc[fc])
                            lb_tiles[fc] = lb_cur
                        bps = base_ps.tile([128, TT], F32, tag="bps")
                        for ci in range(DCH):
                            nc.tensor.matmul(
                                bps, lhsT=wi_cur[:, ci, :],
                                rhs=hsB_sb[:, ci, tsl],
                                start=(ci == 0), stop=(ci == DCH - 1))
                        bs = bs_pool.tile([128, TT], BF16, tag="bs")
                        nc.scalar.copy(out=bs, in_=bps)
                        pend[step] = (fc, tt, tsl, bs)
                        if step == 44:
                            for fcp in range(WO_PRE):
                                wo_fetch(0, fcp)
                    j = step - LAG
                    if j in pend:
                        fcj, ttj, tslj, bsj = pend.pop(j)
                        bic = bi_sb[:, fcj:fcj + 1]
                        d1 = d_ps.tile([128, TT], F32, tag="d")
                        nc.tensor.matmul(d1, lhsT=lb_tiles[fcj],
                                         rhs=tA1_sb[:, tslj],
                                         start=True, stop=True)
                        d2 = d_ps.tile([128, TT], F32, tag="d")
                        nc.tensor.matmul(d2, lhsT=lb_tiles[fcj],
                                         rhs=tA2_sb[:, tslj],
                                         start=True, stop=True)
                        s1 = s_pool.tile([128, TT], BF16, tag="s")
                        nc.vector.tensor_tensor(out=s1, in0=d1, in1=bsj,
                                                op=ALU.add)
                        r1 = r_pool.tile([128, TT], BF16, tag="r")
                        nc.scalar.activation(r1, s1, AF.Relu, bias=bic)
                        s2 = s_pool.tile([128, TT], BF16, tag="s")
                        nc.vector.tensor_tensor(out=s2, in0=d2, in1=bsj,
                                                op=ALU.add)
                        r2 = r_pool.tile([128, TT], BF16, tag="r")
                        nc.scalar.activation(r2, s2, AF.Relu, bias=bic)
                        nc.vector.tensor_tensor(
                            out=H_sb[fcj][:, tslj], in0=r1, in1=r2, op=ALU.add)

        # ---- phase 3: wo matmul, out = (psum + 2*bo) * v ----
        with (
            tc.tile_pool(name="wo_ps", bufs=8, space="PSUM") as wo_ps,
            tc.tile_pool(name="o_sb", bufs=3) as o_pool,
        ):
            for tt in range(NTT):
                tsl = slice(tt * TT, (tt + 1) * TT)
                ops = [wo_ps.tile([128, TT], F32, tag="wops",
                                  name=f"o{tt}_{dc}") for dc in range(DCH)]
                for fc in range(FCH):
                    if (tt, fc) not in wo_tiles:
                        wo_fetch(tt, fc)
                    wo_t = wo_tiles.pop((tt, fc))
                    if tt == 0 and fc == 28:
                        for fcp in range(4):
                            wo_fetch(1, fcp)
                    for dc in range(DCH):
                        nc.tensor.matmul(
                            ops[dc], lhsT=wo_t[:, dc * 128:(dc + 1) * 128],
                            rhs=H_sb[fc][:, tsl],
                            start=(fc == 0), stop=(fc == FCH - 1))
                for dc in range(DCH):
                    # split drain: ACT does psum+2bo (per-partition bias) to
                    # bf16, DVE does the per-token *v -- halves the end tail
                    o_a = o_pool.tile([128, TT], BF16, tag="oa")
                    nc.scalar.activation(o_a, ops[dc], AF.Identity,
                                         bias=bo2_sb[:, dc:dc + 1])
                    o_t = o_pool.tile([128, TT], BF16, tag="ot")
                    nc.vector.tensor_tensor(out=o_t, in0=o_a,
                                            in1=Vb_sb[:, tsl], op=ALU.mult)
                    nc.sync.dma_start(
                        out=outT[dc * 128:(dc + 1) * 128, tsl], in_=o_t)

    nc.compile()
    return nc


def prep_inputs(hidden_states, wi, bi, wo, bo, lora_A, lora_B, gate_w, gate_b):
    """Host-side layout prep; returns per-core input maps."""
    import ml_dtypes
    bf = ml_dtypes.bfloat16
    f32 = np.float32
    hs = np.asarray(hidden_states, f32).reshape(TOK, DM)
    wi = np.asarray(wi, f32); bi = np.asarray(bi, f32)
    wo = np.asarray(wo, f32); bo = np.asarray(bo, f32)
    lora_A = np.asarray(lora_A, f32); lora_B = np.asarray(lora_B, f32)
    gate_w = np.asarray(gate_w, f32); gate_b = np.asarray(gate_b, f32)

    # wi^T in (fc, k, ci, m) bf16 chunks
    wiB = np.ascontiguousarray(
        wi.T.reshape(DCH, 128, FCH, 128).transpose(2, 1, 0, 3)).astype(bf)
    biC = np.ascontiguousarray(bi.reshape(FCH, 128).T)

    # lora-A concatenated: columns 16e+r = lora_A[e,r,:]
    lA_cat = np.concatenate([lora_A[e].T for e in range(E)], axis=1)  # [DM,128]
    lAc = np.ascontiguousarray(
        lA_cat.reshape(DCH, 128, 128).transpose(1, 0, 2)).astype(bf)
    # lora-B concatenated: rows 16e+r = lora_B[e,:,r]
    lB_cat = np.concatenate([lora_B[e].T for e in range(E)], axis=0)  # [128,FF]
    lBc = np.ascontiguousarray(
        lB_cat.reshape(128, FCH, 128).transpose(1, 0, 2)).astype(bf)

    gwB = np.ascontiguousarray(
        gate_w.T.reshape(DCH, 128, E).transpose(1, 0, 2)).astype(bf)
    gbC = np.ascontiguousarray(gate_b.reshape(E, 1))

    woB = np.ascontiguousarray(wo.T.reshape(FCH, 128, DM)).astype(bf)
    bo2 = np.ascontiguousarray((2.0 * bo).reshape(DCH, 128).T)       # [128,8]
    gG = np.kron(np.eye(E, dtype=f32), np.ones((1, 16), f32)).astype(bf)

    shared = dict(wiB=wiB, biC=biC, lAc=lAc, lBc=lBc, gwB=gwB, gbC=gbC,
                  woB=woB, bo2=bo2, gG=gG)
    in_maps = []
    for c in range(NCORES):
        hsc = hs[c * T:(c + 1) * T]                                  # [T, DM]
        hsB = np.ascontiguousarray(
            hsc.T.reshape(DCH, 128, T).transpose(1, 0, 2)).astype(bf)
        in_maps.append(dict(hsB=hsB, **shared))
    return in_maps


def run(in_maps, **kwargs):
    nc = build_bass()
    return nc, run_bass_kernel_spmd(nc, in_maps, list(range(NCORES)), **kwargs)


def kernel(hidden_states, wi, bi, wo, bo, lora_A, lora_B, gate_w, gate_b):
    in_maps = prep_inputs(hidden_states, wi, bi, wo, bo, lora_A, lora_B,
                          gate_w, gate_b)
    _, res = run(in_maps)
    out = np.stack([res.results[c]["outT"].T for c in range(NCORES)])
    return out.reshape(B, S, DM).astype(np.float32)


# revision 27
# speedup vs baseline: 1.0366x; 1.0084x over previous
"""Trainium2 Bass kernel for the MoE block (nn_MoEBlock_5592047420171).

Strategy: data-parallel over tokens across 8 NeuronCores (1024 tokens/core,
all weights replicated; no collectives).  Layout: d_ff on partitions, tokens
on the free dim.

  out[t,:] = v_t * (relu(base_t + bi + delta_{e1,t}) + relu(base_t + bi +
             delta_{e2,t})) @ wo^T + 2 v_t * bo

Design (v3):
  * router logits in bf16 (f32 PSUM accum); exp with gate_b as ACT bias in
    [8,T] layout; top-2 / one-hots via gpsimd partition_all_reduce(max) and
    DVE is_equal -- no PE transposes.  Processed in two token halves so the
    whole chain pipelines across engines; masks expanded to the concat-lora
    layout by a DRAM round-trip gather (baseline idiom).
  * lora path bf16: tA = hs@lA_cat (one [128,T] concat matmul), masked per
    top-2 choice by a DVE multiply straight out of PSUM; delta = lB_cat@tA_k
    is one K=128 matmul per choice per tile.
  * base = hs@wi^T in bf16, drained to bf16 by ACT copy (no bias); relu with
    bi as per-partition ACT bias; H = r1+r2 (DVE); v-scaling moved to the
    output drain: out = (wo_psum + 2*bo) * V (one DVE op per out tile).
  * main loop software-pipelined: deltas+combine lag LAG tiles behind base
    matmuls so the router/mask chain hides under base compute and the PE
    never idles (HAM throttling avoided).  wo tiles prefetched during the
    main-loop tail, streamed per-fc (re-streamed per token-half).
"""

import numpy as np
from contextlib import ExitStack

import concourse.bass as bass
import concourse.tile as tile
from concourse import bacc, mybir, bass_isa
from concourse.bass_utils import run_bass_kernel_spmd

B, S, DM, FF, E, RK = 4, 2048, 1024, 4096, 8, 16
NCORES = 8
TOK = B * S            # 8192 tokens
T = TOK // NCORES      # 1024 tokens per core
TT = 512               # token tile width (free dim of big matmuls)
NTT = T // TT          # 2 token tiles
DCH = 8                # d_model chunks of 128
FCH = FF // 128        # 32 d_ff chunks of 128
LAG = 7                # pipeline lag (tiles) between base and delta stages
WO_PRE = 8             # wo tiles prefetched during the main loop

F32 = mybir.dt.float32
BF16 = mybir.dt.bfloat16
ALU = mybir.AluOpType
AF = mybir.ActivationFunctionType
ROP = bass_isa.ReduceOp


def build_bass():
    nc = bacc.Bacc("TRN2", target_bir_lowering=False)

    hsB = nc.declare_dram_parameter("hsB", [128, DCH, T], BF16, isOutput=False)
    gwB = nc.declare_dram_parameter("gwB", [128, DCH, 8], BF16, isOutput=False)
    gbC = nc.declare_dram_parameter("gbC", [8, 1], F32, isOutput=False)
    wiB = nc.declare_dram_parameter("wiB", [FCH, 128, DCH, 128], BF16, isOutput=False)
    biC = nc.declare_dram_parameter("biC", [128, FCH], F32, isOutput=False)
    lAc = nc.declare_dram_parameter("lAc", [128, DCH, 128], BF16, isOutput=False)
    lBc = nc.declare_dram_parameter("lBc", [FCH, 128, 128], BF16, isOutput=False)
    woB = nc.declare_dram_parameter("woB", [FCH, 128, DM], BF16, isOutput=False)
    bo2 = nc.declare_dram_parameter("bo2", [128, 8], F32, isOutput=False)
    gG = nc.declare_dram_parameter("gG", [8, 128], BF16, isOutput=False)
    outT = nc.declare_dram_parameter("outT", [DM, T], BF16, isOutput=True)

    hsB, gwB, gbC, wiB, biC, lAc, lBc, woB, bo2, gG, outT = (
        h.ap() for h in (hsB, gwB, gbC, wiB, biC, lAc, lBc, woB, bo2, gG, outT))

    with tile.TileContext(nc) as tc, ExitStack() as ctx:
        persist = ctx.enter_context(tc.tile_pool(name="persist", bufs=1))

        # ---- resident tensors (DMA order = priority order) ----
        gw_sb = persist.tile([128, DCH, 8], BF16, tag="gw")
        nc.sync.dma_start(out=gw_sb, in_=gwB)
        gb_sb = persist.tile([8, 1], F32, tag="gb")
        nc.sync.dma_start(out=gb_sb, in_=gbC)
        hsB_sb = persist.tile([128, DCH, T], BF16, tag="hsB")
        for h in range(NTT):
            hsl = slice(h * TT, (h + 1) * TT)
            nc.sync.dma_start(out=hsB_sb[:, :, hsl], in_=hsB[:, :, hsl])
        lA_sb = persist.tile([128, DCH, 128], BF16, tag="lA")
        nc.sync.dma_start(out=lA_sb, in_=lAc)
        bi_sb = persist.tile([128, FCH], F32, tag="bi")
        nc.sync.dma_start(out=bi_sb, in_=biC)
        bo2_sb = persist.tile([128, 8], F32, tag="bo2")
        nc.sync.dma_start(out=bo2_sb, in_=bo2)
        gG_sb = persist.tile([8, 128], BF16, tag="gG")
        nc.sync.dma_start(out=gG_sb, in_=gG)

        # ---- router state (SBUF) ----
        P_sb = persist.tile([8, T], F32, tag="P")
        m1_sb = persist.tile([8, T], F32, tag="m1")
        q_sb = persist.tile([8, T], F32, tag="q")
        m2_sb = persist.tile([8, T], F32, tag="m2")
        S_sb = persist.tile([8, T], F32, tag="S")
        oh1_sb = persist.tile([8, T], BF16, tag="oh1")
        oh2_sb = persist.tile([8, T], BF16, tag="oh2")
        rv_sb = persist.tile([1, T], F32, tag="rv")
        vn_sb = persist.tile([1, T], F32, tag="vn")
        v_sb = persist.tile([1, T], F32, tag="v")
        Vb_sb = persist.tile([128, T], F32, tag="Vb")
        M1_sb = persist.tile([128, T], BF16, tag="M1")
        M2_sb = persist.tile([128, T], BF16, tag="M2")
        tA1_sb = persist.tile([128, T], BF16, tag="tA1")
        tA2_sb = persist.tile([128, T], BF16, tag="tA2")
        H_sb = [persist.tile([128, T], BF16, tag=f"H{fc}", name=f"H{fc}")
                for fc in range(FCH)]

        wo_pool = ctx.enter_context(tc.tile_pool(name="wo_sb", bufs=WO_PRE + 6))
        wo_tiles = {}

        def wo_fetch(tt, fc):
            wo_t = wo_pool.tile([128, DM], BF16, tag="wo", name=f"wo{tt}_{fc}")
            nc.sync.dma_start(out=wo_t, in_=woB[fc])
            wo_tiles[(tt, fc)] = wo_t

        with tc.tile_pool(name="base_ps", bufs=2, space="PSUM") as base_ps:
            # ---- phase 1: router + lora-A, pipelined per token half ----
            with (
                tc.tile_pool(name="rt_ps", bufs=1, space="PSUM") as rt_ps,
                tc.tile_pool(name="la_ps", bufs=2, space="PSUM") as la_ps,
                tc.tile_pool(name="msk_ps", bufs=2, space="PSUM") as msk_ps,
            ):
                for h in range(NTT):
                    hsl = slice(h * TT, (h + 1) * TT)
                    lg = rt_ps.tile([8, TT], F32, tag="lg")
                    for ci in range(DCH):
                        nc.tensor.matmul(
                            lg, lhsT=gw_sb[:, ci, :], rhs=hsB_sb[:, ci, hsl],
                            start=(ci == 0), stop=(ci == DCH - 1))
                    nc.scalar.activation(P_sb[:, hsl], lg, AF.Exp, bias=gb_sb)
                    # top-1/top-2 via partition all-reduce max + is_equal
                    nc.gpsimd.partition_all_reduce(
                        m1_sb[:, hsl], P_sb[:, hsl], channels=8,
                        reduce_op=ROP.max)
                    nc.vector.tensor_tensor(
                        out=oh1_sb[:, hsl], in0=P_sb[:, hsl],
                        in1=m1_sb[:, hsl], op=ALU.is_equal)
                    nc.vector.scalar_tensor_tensor(
                        out=q_sb[:, hsl], in0=oh1_sb[:, hsl], scalar=-1e30,
                        in1=P_sb[:, hsl], op0=ALU.mult, op1=ALU.add)
                    nc.gpsimd.partition_all_reduce(
                        m2_sb[:, hsl], q_sb[:, hsl], channels=8,
                        reduce_op=ROP.max)
                    nc.vector.tensor_tensor(
                        out=oh2_sb[:, hsl], in0=q_sb[:, hsl],
                        in1=m2_sb[:, hsl], op=ALU.is_equal)
                    # expand one-hots to concat-row masks on the PE:
                    # M = G @ oh where G replicates expert row e to rows
                    # 16e..16e+15 (exact for 0/1 values)
                    # wait-hint: keep these tiny PE matmuls from being
                    # scheduled before the one-hot chain has resolved (they
                    # would stall the in-order PE stream)
                    with tc.tile_wait_until(0.028 + 0.004 * h):
                        M1p = msk_ps.tile([128, TT], F32, tag="Mp",
                                          name=f"M1{h}")
                        nc.tensor.matmul(M1p, lhsT=gG_sb, rhs=oh1_sb[:, hsl],
                                         start=True, stop=True)
                        nc.scalar.copy(out=M1_sb[:, hsl], in_=M1p)
                        M2p = msk_ps.tile([128, TT], F32, tag="Mp",
                                          name=f"M2{h}")
                        nc.tensor.matmul(M2p, lhsT=gG_sb, rhs=oh2_sb[:, hsl],
                                         start=True, stop=True)
                        nc.scalar.copy(out=M2_sb[:, hsl], in_=M2p)
                    # lora-A for this half + per-choice masking
                    pta = la_ps.tile([128, TT], F32, tag="pta")
                    for ci in range(DCH):
                        nc.tensor.matmul(
                            pta, lhsT=lA_sb[:, ci, :], rhs=hsB_sb[:, ci, hsl],
                            start=(ci == 0), stop=(ci == DCH - 1))
                    nc.vector.tensor_tensor(
                        out=tA1_sb[:, hsl], in0=pta, in1=M1_sb[:, hsl],
                        op=ALU.mult)
                    nc.vector.tensor_tensor(
                        out=tA2_sb[:, hsl], in0=pta, in1=M2_sb[:, hsl],
                        op=ALU.mult)

                # val_sum v = (m1 + m2) / sum(P): only needed by the phase-3
                # drains -- hint the scheduler to keep it off the router
                # critical path (the [1,T] reciprocal alone is ~6.5us on DVE)
                with tc.tile_wait_until(0.08):
                    nc.gpsimd.partition_all_reduce(S_sb, P_sb, channels=8,
                                                   reduce_op=ROP.add)
                    nc.vector.reciprocal_approx_fast(rv_sb, S_sb[0:1, :])
                    nc.vector.tensor_tensor(out=vn_sb, in0=m1_sb[0:1, :],
                                            in1=m2_sb[0:1, :], op=ALU.add)
                    nc.vector.tensor_tensor(out=v_sb, in0=vn_sb,
                                            in1=rv_sb, op=ALU.mult)
                    nc.gpsimd.partition_broadcast(Vb_sb, v_sb, channels=128)

            # ---- phase 2: main loop (base + delta + combine), pipelined ----
            with (
                tc.tile_pool(name="d_ps", bufs=4, space="PSUM") as d_ps,
                tc.tile_pool(name="wi_sb", bufs=6) as wi_pool,
                tc.tile_pool(name="lb_sb", bufs=8) as lb_pool,
                tc.tile_pool(name="bs_sb", bufs=LAG + 3) as bs_pool,
                tc.tile_pool(name="s_sb", bufs=3) as s_pool,
                tc.tile_pool(name="r_sb", bufs=3) as r_pool,
            ):
                tiles = [(fc, tt) for fc in range(FCH) for tt in range(NTT)]
                wi_cur = None
                lb_tiles = {}
                pend = {}
                for step in range(len(tiles) + LAG):
                    if step < len(tiles):
                        fc, tt = tiles[step]
                        tsl = slice(tt * TT, (tt + 1) * TT)
                        if tt == 0:
                            wi_cur = wi_pool.tile([128, DCH, 128], BF16,
                                                  tag="wi", name=f"wi{fc}")
                            nc.sync.dma_start(out=wi_cur, in_=wiB[fc])
                            lb_cur = lb_pool.tile([128, 128], BF16,
                                                  tag="lb", name=f"lb{fc}")
                            nc.sync.dma_start(out=lb_cur, in_=lBc[fc])
                            lb_tiles[fc] = lb_cur
                        bps = base_ps.tile([128, TT], F32, tag="bps")
                        for ci in range(DCH):
                            nc.tensor.matmul(
                                bps, lhsT=wi_cur[:, ci, :],
                                rhs=hsB_sb[:, ci, tsl],
                                start=(ci == 0), stop=(ci == DCH - 1))
                        bs = bs_pool.tile([128, TT], BF16, tag="bs")
                        nc.scalar.copy(out=bs, in_=bps)
                        pend[step] = (fc, tt, tsl, bs)
                        if step == 44:
                            for fcp in range(WO_PRE):
                                wo_fetch(0, fcp)
                    j = step - LAG
                    if j in pend:
                        fcj, ttj, tslj, bsj = pend.pop(j)
                        bic = bi_sb[:, fcj:fcj + 1]
                        d1 = d_ps.tile([128, TT], F32, tag="d")
                        nc.tensor.matmul(d1, lhsT=lb_tiles[fcj],
                                         rhs=tA1_sb[:, tslj],
                                         start=True, stop=True)
                        d2 = d_ps.tile([128, TT], F32, tag="d")
                        nc.tensor.matmul(d2, lhsT=lb_tiles[fcj],
                                         rhs=tA2_sb[:, tslj],
                                         start=True, stop=True)
                        s1 = s_pool.tile([128, TT], BF16, tag="s")
                        nc.vector.tensor_tensor(out=s1, in0=d1, in1=bsj,
                                                op=ALU.add)
                        r1 = r_pool.tile([128, TT], BF16, tag="r")
                        nc.scalar.activation(r1, s1, AF.Relu, bias=bic)
                        s2 = s_pool.tile([128, TT], BF16, tag="s")
                        nc.vector.tensor_tensor(out=s2, in0=d2, in1=bsj,
                                                op=ALU.add)
                        r2 = r_pool.tile([128, TT], BF16, tag="r")
                        nc.scalar.activation(r2, s2, AF.Relu, bias=bic)
                        nc.vector.tensor_tensor(
                            out=H_sb[fcj][:, tslj], in0=r1, in1=r2, op=ALU.add)

        # ---- phase 3: wo matmul, out = (psum + 2*bo) * v ----
        with (
            tc.tile_pool(name="wo_ps", bufs=8, space="PSUM") as wo_ps,
            tc.tile_pool(name="o_sb", bufs=3) as o_pool,
        ):
            for tt in range(NTT):
                tsl = slice(tt * TT, (tt + 1) * TT)
                ops = [wo_ps.tile([128, TT], F32, tag="wops",
                                  name=f"o{tt}_{dc}") for dc in range(DCH)]
                for fc in range(FCH):
                    if (tt, fc) not in wo_tiles:
                        wo_fetch(tt, fc)
                    wo_t = wo_tiles.pop((tt, fc))
                    if tt == 0 and fc == 26:
                        for fcp in range(6):
                            wo_fetch(1, fcp)
                    for dc in range(DCH):
                        nc.tensor.matmul(
                            ops[dc], lhsT=wo_t[:, dc * 128:(dc + 1) * 128],
                            rhs=H_sb[fc][:, tsl],
                            start=(fc == 0), stop=(fc == FCH - 1))
                for dc in range(DCH):
                    # split drain: ACT does psum+2bo (per-partition bias) to
                    # bf16, DVE does the per-token *v -- halves the end tail
                    o_a = o_pool.tile([128, TT], BF16, tag="oa")
                    nc.scalar.activation(o_a, ops[dc], AF.Identity,
                                         bias=bo2_sb[:, dc:dc + 1])
                    o_t = o_pool.tile([128, TT], BF16, tag="ot")
                    nc.vector.tensor_tensor(out=o_t, in0=o_a,
                                            in1=Vb_sb[:, tsl], op=ALU.mult)
                    nc.sync.dma_start(
                        out=outT[dc * 128:(dc + 1) * 128, tsl], in_=o_t)

    nc.compile()
    return nc


def prep_inputs(hidden_states, wi, bi, wo, bo, lora_A, lora_B, gate_w, gate_b):
    """Host-side layout prep; returns per-core input maps."""
    import ml_dtypes
    bf = ml_dtypes.bfloat16
    f32 = np.float32
    hs = np.asarray(hidden_states, f32).reshape(TOK, DM)
    wi = np.asarray(wi, f32); bi = np.asarray(bi, f32)
    wo = np.asarray(wo, f32); bo = np.asarray(bo, f32)
    lora_A = np.asarray(lora_A, f32); lora_B = np.asarray(lora_B, f32)
    gate_w = np.asarray(gate_w, f32); gate_b = np.asarray(gate_b, f32)

    # wi^T in (fc, k, ci, m) bf16 chunks
    wiB = np.ascontiguousarray(
        wi.T.reshape(DCH, 128, FCH, 128).transpose(2, 1, 0, 3)).astype(bf)
    biC = np.ascontiguousarray(bi.reshape(FCH, 128).T)

    # lora-A concatenated: columns 16e+r = lora_A[e,r,:]
    lA_cat = np.concatenate([lora_A[e].T for e in range(E)], axis=1)  # [DM,128]
    lAc = np.ascontiguousarray(
        lA_cat.reshape(DCH, 128, 128).transpose(1, 0, 2)).astype(bf)
    # lora-B concatenated: rows 16e+r = lora_B[e,:,r]
    lB_cat = np.concatenate([lora_B[e].T for e in range(E)], axis=0)  # [128,FF]
    lBc = np.ascontiguousarray(
        lB_cat.reshape(128, FCH, 128).transpose(1, 0, 2)).astype(bf)

    gwB = np.ascontiguousarray(
        gate_w.T.reshape(DCH, 128, E).transpose(1, 0, 2)).astype(bf)
    gbC = np.ascontiguousarray(gate_b.reshape(E, 1))

    woB = np.ascontiguousarray(wo.T.reshape(FCH, 128, DM)).astype(bf)
    bo2 = np.ascontiguousarray((2.0 * bo).reshape(DCH, 128).T)       # [128,8]
    gG = np.kron(np.eye(E, dtype=f32), np.ones((1, 16), f32)).astype(bf)

    shared = dict(wiB=wiB, biC=biC, lAc=lAc, lBc=lBc, gwB=gwB, gbC=gbC,
                  woB=woB, bo2=bo2, gG=gG)
    in_maps = []
    for c in range(NCORES):
        hsc = hs[c * T:(c + 1) * T]                                  # [T, DM]
        hsB = np.ascontiguousarray(
            hsc.T.reshape(DCH, 128, T).transpose(1, 0, 2)).astype(bf)
        in_maps.append(dict(hsB=hsB, **shared))
    return in_maps


def run(in_maps, **kwargs):
    nc = build_bass()
    return nc, run_bass_kernel_spmd(nc, in_maps, list(range(NCORES)), **kwargs)


def kernel(hidden_states, wi, bi, wo, bo, lora_A, lora_B, gate_w, gate_b):
    in_maps = prep_inputs(hidden_states, wi, bi, wo, bo, lora_A, lora_B,
                          gate_w, gate_b)
    _, res = run(in_maps)
    out = np.stack([res.results[c]["outT"].T for c in range(NCORES)])
    return out.reshape(B, S, DM).astype(np.float32)


# revision 28
# speedup vs baseline: 1.0381x; 1.0015x over previous
"""Trainium2 Bass kernel for the MoE block (nn_MoEBlock_5592047420171).

Strategy: data-parallel over tokens across 8 NeuronCores (1024 tokens/core,
all weights replicated; no collectives).  Layout: d_ff on partitions, tokens
on the free dim.

  out[t,:] = v_t * (relu(base_t + bi + delta_{e1,t}) + relu(base_t + bi +
             delta_{e2,t})) @ wo^T + 2 v_t * bo

Design (v3):
  * router logits in bf16 (f32 PSUM accum); exp with gate_b as ACT bias in
    [8,T] layout; top-2 / one-hots via gpsimd partition_all_reduce(max) and
    DVE is_equal -- no PE transposes.  Processed in two token halves so the
    whole chain pipelines across engines; masks expanded to the concat-lora
    layout by a DRAM round-trip gather (baseline idiom).
  * lora path bf16: tA = hs@lA_cat (one [128,T] concat matmul), masked per
    top-2 choice by a DVE multiply straight out of PSUM; delta = lB_cat@tA_k
    is one K=128 matmul per choice per tile.
  * base = hs@wi^T in bf16, drained to bf16 by ACT copy (no bias); relu with
    bi as per-partition ACT bias; H = r1+r2 (DVE); v-scaling moved to the
    output drain: out = (wo_psum + 2*bo) * V (one DVE op per out tile).
  * main loop software-pipelined: deltas+combine lag LAG tiles behind base
    matmuls so the router/mask chain hides under base compute and the PE
    never idles (HAM throttling avoided).  wo tiles prefetched during the
    main-loop tail, streamed per-fc (re-streamed per token-half).
"""

import numpy as np
from contextlib import ExitStack

import concourse.bass as bass
import concourse.tile as tile
from concourse import bacc, mybir, bass_isa
from concourse.bass_utils import run_bass_kernel_spmd

B, S, DM, FF, E, RK = 4, 2048, 1024, 4096, 8, 16
NCORES = 8
TOK = B * S            # 8192 tokens
T = TOK // NCORES      # 1024 tokens per core
TT = 512               # token tile width (free dim of big matmuls)
NTT = T // TT          # 2 token tiles
DCH = 8                # d_model chunks of 128
FCH = FF // 128        # 32 d_ff chunks of 128
LAG = 7                # pipeline lag (tiles) between base and delta stages
WO_PRE = 8             # wo tiles prefetched during the main loop

F32 = mybir.dt.float32
BF16 = mybir.dt.bfloat16
ALU = mybir.AluOpType
AF = mybir.ActivationFunctionType
ROP = bass_isa.ReduceOp


def build_bass():
    nc = bacc.Bacc("TRN2", target_bir_lowering=False)

    hsB = nc.declare_dram_parameter("hsB", [128, DCH, T], BF16, isOutput=False)
    gwB = nc.declare_dram_parameter("gwB", [128, DCH, 8], BF16, isOutput=False)
    gbC = nc.declare_dram_parameter("gbC", [8, 1], F32, isOutput=False)
    wiB = nc.declare_dram_parameter("wiB", [FCH, 128, DCH, 128], BF16, isOutput=False)
    biC = nc.declare_dram_parameter("biC", [128, FCH], F32, isOutput=False)
    lAc = nc.declare_dram_parameter("lAc", [128, DCH, 128], BF16, isOutput=False)
    lBc = nc.declare_dram_parameter("lBc", [FCH, 128, 128], BF16, isOutput=False)
    woB = nc.declare_dram_parameter("woB", [FCH, 128, DM], BF16, isOutput=False)
    bo2 = nc.declare_dram_parameter("bo2", [128, 8], F32, isOutput=False)
    gG = nc.declare_dram_parameter("gG", [8, 128], BF16, isOutput=False)
    outT = nc.declare_dram_parameter("outT", [DM, T], BF16, isOutput=True)

    hsB, gwB, gbC, wiB, biC, lAc, lBc, woB, bo2, gG, outT = (
        h.ap() for h in (hsB, gwB, gbC, wiB, biC, lAc, lBc, woB, bo2, gG, outT))

    with tile.TileContext(nc) as tc, ExitStack() as ctx:
        persist = ctx.enter_context(tc.tile_pool(name="persist", bufs=1))

        # ---- resident tensors (DMA order = priority order) ----
        gw_sb = persist.tile([128, DCH, 8], BF16, tag="gw")
        nc.sync.dma_start(out=gw_sb, in_=gwB)
        gb_sb = persist.tile([8, 1], F32, tag="gb")
        nc.sync.dma_start(out=gb_sb, in_=gbC)
        hsB_sb = persist.tile([128, DCH, T], BF16, tag="hsB")
        for h in range(NTT):
            hsl = slice(h * TT, (h + 1) * TT)
            nc.sync.dma_start(out=hsB_sb[:, :, hsl], in_=hsB[:, :, hsl])
        lA_sb = persist.tile([128, DCH, 128], BF16, tag="lA")
        nc.sync.dma_start(out=lA_sb, in_=lAc)
        bi_sb = persist.tile([128, FCH], F32, tag="bi")
        nc.sync.dma_start(out=bi_sb, in_=biC)
        bo2_sb = persist.tile([128, 8], F32, tag="bo2")
        nc.sync.dma_start(out=bo2_sb, in_=bo2)
        gG_sb = persist.tile([8, 128], BF16, tag="gG")
        nc.sync.dma_start(out=gG_sb, in_=gG)

        # ---- PE warmup: dummy matmuls on a zeroed tile while the input
        # DMAs land, so the tensor engine is at full p-state (no HAM cold
        # tax) when the real work starts ----
        warm_sb = persist.tile([128, TT], BF16, tag="warm")
        nc.vector.memset(warm_sb, 0.0)
        with tc.tile_pool(name="warm_ps", bufs=1, space="PSUM") as warm_ps:
            wps = warm_ps.tile([128, TT], F32, tag="wps")
            for _ in range(24):
                nc.tensor.matmul(wps, lhsT=warm_sb[:, 0:128], rhs=warm_sb,
                                 start=True, stop=True)

        # ---- router state (SBUF) ----
        P_sb = persist.tile([8, T], F32, tag="P")
        m1_sb = persist.tile([8, T], F32, tag="m1")
        q_sb = persist.tile([8, T], F32, tag="q")
        m2_sb = persist.tile([8, T], F32, tag="m2")
        S_sb = persist.tile([8, T], F32, tag="S")
        oh1_sb = persist.tile([8, T], BF16, tag="oh1")
        oh2_sb = persist.tile([8, T], BF16, tag="oh2")
        rv_sb = persist.tile([1, T], F32, tag="rv")
        vn_sb = persist.tile([1, T], F32, tag="vn")
        v_sb = persist.tile([1, T], F32, tag="v")
        Vb_sb = persist.tile([128, T], F32, tag="Vb")
        M1_sb = persist.tile([128, T], BF16, tag="M1")
        M2_sb = persist.tile([128, T], BF16, tag="M2")
        tA1_sb = persist.tile([128, T], BF16, tag="tA1")
        tA2_sb = persist.tile([128, T], BF16, tag="tA2")
        H_sb = [persist.tile([128, T], BF16, tag=f"H{fc}", name=f"H{fc}")
                for fc in range(FCH)]

        wo_pool = ctx.enter_context(tc.tile_pool(name="wo_sb", bufs=WO_PRE + 6))
        wo_tiles = {}

        def wo_fetch(tt, fc):
            wo_t = wo_pool.tile([128, DM], BF16, tag="wo", name=f"wo{tt}_{fc}")
            nc.sync.dma_start(out=wo_t, in_=woB[fc])
            wo_tiles[(tt, fc)] = wo_t

        with tc.tile_pool(name="base_ps", bufs=2, space="PSUM") as base_ps:
            # ---- phase 1: router + lora-A, pipelined per token half ----
            with (
                tc.tile_pool(name="rt_ps", bufs=1, space="PSUM") as rt_ps,
                tc.tile_pool(name="la_ps", bufs=2, space="PSUM") as la_ps,
                tc.tile_pool(name="msk_ps", bufs=2, space="PSUM") as msk_ps,
            ):
                for h in range(NTT):
                    hsl = slice(h * TT, (h + 1) * TT)
                    lg = rt_ps.tile([8, TT], F32, tag="lg")
                    for ci in range(DCH):
                        nc.tensor.matmul(
                            lg, lhsT=gw_sb[:, ci, :], rhs=hsB_sb[:, ci, hsl],
                            start=(ci == 0), stop=(ci == DCH - 1))
                    nc.scalar.activation(P_sb[:, hsl], lg, AF.Exp, bias=gb_sb)
                    # top-1/top-2 via partition all-reduce max + is_equal
                    nc.gpsimd.partition_all_reduce(
                        m1_sb[:, hsl], P_sb[:, hsl], channels=8,
                        reduce_op=ROP.max)
                    nc.vector.tensor_tensor(
                        out=oh1_sb[:, hsl], in0=P_sb[:, hsl],
                        in1=m1_sb[:, hsl], op=ALU.is_equal)
                    nc.vector.scalar_tensor_tensor(
                        out=q_sb[:, hsl], in0=oh1_sb[:, hsl], scalar=-1e30,
                        in1=P_sb[:, hsl], op0=ALU.mult, op1=ALU.add)
                    nc.gpsimd.partition_all_reduce(
                        m2_sb[:, hsl], q_sb[:, hsl], channels=8,
                        reduce_op=ROP.max)
                    nc.vector.tensor_tensor(
                        out=oh2_sb[:, hsl], in0=q_sb[:, hsl],
                        in1=m2_sb[:, hsl], op=ALU.is_equal)
                    # expand one-hots to concat-row masks on the PE:
                    # M = G @ oh where G replicates expert row e to rows
                    # 16e..16e+15 (exact for 0/1 values)
                    # wait-hint: keep these tiny PE matmuls from being
                    # scheduled before the one-hot chain has resolved (they
                    # would stall the in-order PE stream)
                    with tc.tile_wait_until(0.028 + 0.004 * h):
                        M1p = msk_ps.tile([128, TT], F32, tag="Mp",
                                          name=f"M1{h}")
                        nc.tensor.matmul(M1p, lhsT=gG_sb, rhs=oh1_sb[:, hsl],
                                         start=True, stop=True)
                        nc.scalar.copy(out=M1_sb[:, hsl], in_=M1p)
                        M2p = msk_ps.tile([128, TT], F32, tag="Mp",
                                          name=f"M2{h}")
                        nc.tensor.matmul(M2p, lhsT=gG_sb, rhs=oh2_sb[:, hsl],
                                         start=True, stop=True)
                        nc.scalar.copy(out=M2_sb[:, hsl], in_=M2p)
                    # lora-A for this half + per-choice masking
                    pta = la_ps.tile([128, TT], F32, tag="pta")
                    for ci in range(DCH):
                        nc.tensor.matmul(
                            pta, lhsT=lA_sb[:, ci, :], rhs=hsB_sb[:, ci, hsl],
                            start=(ci == 0), stop=(ci == DCH - 1))
                    nc.vector.tensor_tensor(
                        out=tA1_sb[:, hsl], in0=pta, in1=M1_sb[:, hsl],
                        op=ALU.mult)
                    nc.vector.tensor_tensor(
                        out=tA2_sb[:, hsl], in0=pta, in1=M2_sb[:, hsl],
                        op=ALU.mult)

                # val_sum v = (m1 + m2) / sum(P): only needed by the phase-3
                # drains -- hint the scheduler to keep it off the router
                # critical path (the [1,T] reciprocal alone is ~6.5us on DVE)
                with tc.tile_wait_until(0.08):
                    nc.gpsimd.partition_all_reduce(S_sb, P_sb, channels=8,
                                                   reduce_op=ROP.add)
                    nc.vector.reciprocal_approx_fast(rv_sb, S_sb[0:1, :])
                    nc.vector.tensor_tensor(out=vn_sb, in0=m1_sb[0:1, :],
                                            in1=m2_sb[0:1, :], op=ALU.add)
                    nc.vector.tensor_tensor(out=v_sb, in0=vn_sb,
                                            in1=rv_sb, op=ALU.mult)
                    nc.gpsimd.partition_broadcast(Vb_sb, v_sb, channels=128)

            # ---- phase 2: main loop (base + delta + combine), pipelined ----
            with (
                tc.tile_pool(name="d_ps", bufs=4, space="PSUM") as d_ps,
                tc.tile_pool(name="wi_sb", bufs=6) as wi_pool,
                tc.tile_pool(name="lb_sb", bufs=8) as lb_pool,
                tc.tile_pool(name="bs_sb", bufs=LAG + 3) as bs_pool,
                tc.tile_pool(name="s_sb", bufs=3) as s_pool,
                tc.tile_pool(name="r_sb", bufs=3) as r_pool,
            ):
                tiles = [(fc, tt) for fc in range(FCH) for tt in range(NTT)]
                wi_cur = None
                lb_tiles = {}
                pend = {}
                for step in range(len(tiles) + LAG):
                    if step < len(tiles):
                        fc, tt = tiles[step]
                        tsl = slice(tt * TT, (tt + 1) * TT)
                        if tt == 0:
                            wi_cur = wi_pool.tile([128, DCH, 128], BF16,
                                                  tag="wi", name=f"wi{fc}")
                            nc.sync.dma_start(out=wi_cur, in_=wiB[fc])
                            lb_cur = lb_pool.tile([128, 128], BF16,
                                                  tag="lb", name=f"lb{fc}")
                            nc.sync.dma_start(out=lb_cur, in_=lBc[fc])
                            lb_tiles[fc] = lb_cur
                        bps = base_ps.tile([128, TT], F32, tag="bps")
                        for ci in range(DCH):
                            nc.tensor.matmul(
                                bps, lhsT=wi_cur[:, ci, :],
                                rhs=hsB_sb[:, ci, tsl],
                                start=(ci == 0), stop=(ci == DCH - 1))
                        bs = bs_pool.tile([128, TT], BF16, tag="bs")
                        nc.scalar.copy(out=bs, in_=bps)
                        pend[step] = (fc, tt, tsl, bs)
                        if step == 44:
                            for fcp in range(WO_PRE):
                                wo_fetch(0, fcp)
                    j = step - LAG
                    if j in pend:
                        fcj, ttj, tslj, bsj = pend.pop(j)
                        bic = bi_sb[:, fcj:fcj + 1]
                        d1 = d_ps.tile([128, TT], F32, tag="d")
                        nc.tensor.matmul(d1, lhsT=lb_tiles[fcj],
                                         rhs=tA1_sb[:, tslj],
                                         start=True, stop=True)
                        d2 = d_ps.tile([128, TT], F32, tag="d")
                        nc.tensor.matmul(d2, lhsT=lb_tiles[fcj],
                                         rhs=tA2_sb[:, tslj],
                                         start=True, stop=True)
                        s1 = s_pool.tile([128, TT], BF16, tag="s")
                        nc.vector.tensor_tensor(out=s1, in0=d1, in1=bsj,
                                                op=ALU.add)
                        r1 = r_pool.tile([128, TT], BF16, tag="r")
                        nc.scalar.activation(r1, s1, AF.Relu, bias=bic)
                        s2 = s_pool.tile([128, TT], BF16, tag="s")
                        nc.vector.tensor_tensor(out=s2, in0=d2, in1=bsj,
                                                op=ALU.add)
                        r2 = r_pool.tile([128, TT], BF16, tag="r")
                        nc.scalar.activation(r2, s2, AF.Relu, bias=bic)
                        nc.vector.tensor_tensor(
                            out=H_sb[fcj][:, tslj], in0=r1, in1=r2, op=ALU.add)

        # ---- phase 3: wo matmul, out = (psum + 2*bo) * v ----
        with (
            tc.tile_pool(name="wo_ps", bufs=8, space="PSUM") as wo_ps,
            tc.tile_pool(name="o_sb", bufs=3) as o_pool,
        ):
            for tt in range(NTT):
                tsl = slice(tt * TT, (tt + 1) * TT)
                ops = [wo_ps.tile([128, TT], F32, tag="wops",
                                  name=f"o{tt}_{dc}") for dc in range(DCH)]
                for fc in range(FCH):
                    if (tt, fc) not in wo_tiles:
                        wo_fetch(tt, fc)
                    wo_t = wo_tiles.pop((tt, fc))
                    if tt == 0 and fc == 26:
                        for fcp in range(6):
                            wo_fetch(1, fcp)
                    for dc in range(DCH):
                        nc.tensor.matmul(
                            ops[dc], lhsT=wo_t[:, dc * 128:(dc + 1) * 128],
                            rhs=H_sb[fc][:, tsl],
                            start=(fc == 0), stop=(fc == FCH - 1))
                for dc in range(DCH):
                    # split drain: ACT does psum+2bo (per-partition bias) to
                    # bf16, DVE does the per-token *v -- halves the end tail
                    o_a = o_pool.tile([128, TT], BF16, tag="oa")
                    nc.scalar.activation(o_a, ops[dc], AF.Identity,
                                         bias=bo2_sb[:, dc:dc + 1])
                    o_t = o_pool.tile([128, TT], BF16, tag="ot")
                    nc.vector.tensor_tensor(out=o_t, in0=o_a,
                                            in1=Vb_sb[:, tsl], op=ALU.mult)
                    nc.sync.dma_start(
                        out=outT[dc * 128:(dc + 1) * 128, tsl], in_=o_t)

    nc.compile()
    return nc


def prep_inputs(hidden_states, wi, bi, wo, bo, lora_A, lora_B, gate_w, gate_b):
    """Host-side layout prep; returns per-core input maps."""
    import ml_dtypes
    bf = ml_dtypes.bfloat16
    f32 = np.float32
    hs = np.asarray(hidden_states, f32).reshape(TOK, DM)
    wi = np.asarray(wi, f32); bi = np.asarray(bi, f32)
    wo = np.asarray(wo, f32); bo = np.asarray(bo, f32)
    lora_A = np.asarray(lora_A, f32); lora_B = np.asarray(lora_B, f32)
    gate_w = np.asarray(gate_w, f32); gate_b = np.asarray(gate_b, f32)

    # wi^T in (fc, k, ci, m) bf16 chunks
    wiB = np.ascontiguousarray(
        wi.T.reshape(DCH, 128, FCH, 128).transpose(2, 1, 0, 3)).astype(bf)
    biC = np.ascontiguousarray(bi.reshape(FCH, 128).T)

    # lora-A concatenated: columns 16e+r = lora_A[e,r,:]
    lA_cat = np.concatenate([lora_A[e].T for e in range(E)], axis=1)  # [DM,128]
    lAc = np.ascontiguousarray(
        lA_cat.reshape(DCH, 128, 128).transpose(1, 0, 2)).astype(bf)
    # lora-B concatenated: rows 16e+r = lora_B[e,:,r]
    lB_cat = np.concatenate([lora_B[e].T for e in range(E)], axis=0)  # [128,FF]
    lBc = np.ascontiguousarray(
        lB_cat.reshape(128, FCH, 128).transpose(1, 0, 2)).astype(bf)

    gwB = np.ascontiguousarray(
        gate_w.T.reshape(DCH, 128, E).transpose(1, 0, 2)).astype(bf)
    gbC = np.ascontiguousarray(gate_b.reshape(E, 1))

    woB = np.ascontiguousarray(wo.T.reshape(FCH, 128, DM)).astype(bf)
    bo2 = np.ascontiguousarray((2.0 * bo).reshape(DCH, 128).T)       # [128,8]
    gG = np.kron(np.eye(E, dtype=f32), np.ones((1, 16), f32)).astype(bf)

    shared = dict(wiB=wiB, biC=biC, lAc=lAc, lBc=lBc, gwB=gwB, gbC=gbC,
                  woB=woB, bo2=bo2, gG=gG)
    in_maps = []
    for c in range(NCORES):
        hsc = hs[c * T:(c + 1) * T]                                  # [T, DM]
        hsB = np.ascontiguousarray(
            hsc.T.reshape(DCH, 128, T).transpose(1, 0, 2)).astype(bf)
        in_maps.append(dict(hsB=hsB, **shared))
    return in_maps


def run(in_maps, **kwargs):
    nc = build_bass()
    return nc, run_bass_kernel_spmd(nc, in_maps, list(range(NCORES)), **kwargs)


def kernel(hidden_states, wi, bi, wo, bo, lora_A, lora_B, gate_w, gate_b):
    in_maps = prep_inputs(hidden_states, wi, bi, wo, bo, lora_A, lora_B,
                          gate_w, gate_b)
    _, res = run(in_maps)
    out = np.stack([res.results[c]["outT"].T for c in range(NCORES)])
    return out.reshape(B, S, DM).astype(np.float32)
